# revision 1
# baseline (speedup 1.0000x reference)
"""GNN message-passing (3x GraphConv + mean-pool + FC + softmax, graph 0 only)
on 8 Trainium2 NeuronCores.

Strategy
--------
Nodes are partitioned contiguously across the 8 cores. Because GraphConv is
linear before the nonlinearity, ``segment_sum(h[src]) @ W_rel`` is computed as
``segment_sum(z[src])`` with ``z = h @ W_rel`` (64-wide), so the random-access
traffic is 256B rows. Each layer:

  1. every core computes ``z`` for its own nodes and the shards are
     AllGather-ed into a replicated DRAM table,
  2. each core dma_gathers the z-rows of its in-edges into per-chunk "slot
     grids" ``[128 dst-nodes, R slots, 64]`` where the stream position of a
     slot encodes the destination node (host-built index tables), so the
     segment sum is a pure free-dim tree fold on the Vector engine,
  3. the aggregate is PE-transposed to feature-major, the root term is added
     (precomputed for layer 1, fused as a PSUM-accumulated matmul for layers
     2/3), ReLU-ed, and the next z chunk is written back node-major.

Only ``probs[0]`` is returned by the reference, so layers 2/3 are pruned to
the 1-hop/2-hop in-neighborhoods of graph-0 nodes (computed at runtime from
the actual inputs; exact, not approximate). Mean-pool partial sums are
AllReduce-d, and every core redundantly computes the final FC + softmax.

The per-core node order puts graph-0 nodes first, then the rest of the 1-hop
set, then everything else, each group sorted by in-degree (descending) so the
slot grids of each 128-node chunk are tightly packed.
"""

import os

import numpy as np

import concourse.bacc as bacc
import concourse.bass as bass
import concourse.mybir as mybir
import concourse.tile as tile
from concourse import library_config
from concourse._compat import cdiv
from concourse.bass_utils import run_bass_kernel_spmd
from concourse.masks import make_identity

NCORES = 8
LO = 32768  # int16 gather index limit per table
F32 = mybir.dt.float32
I16 = mybir.dt.int16
AX = mybir.AluOpType
ACTF = mybir.ActivationFunctionType


# ----------------------------------------------------------------------------
# Host-side planning
# ----------------------------------------------------------------------------

def _group_rank_desc(keys: np.ndarray) -> np.ndarray:
    """Stable order (indices) sorting by key descending."""
    return np.argsort(-keys, kind="stable")


class Plan:
    pass


def build_plan(x, edge_index, batch):
    """All host-side index crunching. Returns a Plan with shared (SPMD)
    compile-time constants and per-core input arrays."""
    p = Plan()
    N, F = x.shape
    src = np.asarray(edge_index[0], dtype=np.int64)
    dst = np.asarray(edge_index[1], dtype=np.int64)
    batch = np.asarray(batch, dtype=np.int64)

    NPC = cdiv(N, NCORES)                      # real nodes per core
    NLOC = cdiv(NPC + 1, 128) * 128            # padded local positions (>=1 fake)
    p.N, p.F, p.NPC, p.NLOC = N, F, NPC, NLOC

    # --- pruning sets -------------------------------------------------------
    in_T0 = batch == 0
    p.n0 = int(in_T0.sum())
    e3 = in_T0[dst]                            # edges feeding layer-3 targets
    in_T2 = in_T0.copy()
    in_T2[src[e3]] = True                      # 1-hop in-neighborhood of T0
    e2 = in_T2[dst]

    # --- degree counts (for ordering) ---------------------------------------
    deg1 = np.bincount(dst, minlength=N)
    deg2 = np.bincount(dst[e2], minlength=N)
    deg3 = np.bincount(dst[e3], minlength=N)

    # --- balanced interleaved node->core assignment -------------------------
    # Global order: T0 (by L3 in-degree desc), T2\T0 (by L2 deg), rest (by
    # L1 deg); node j of the global order goes to core j%NCORES at local
    # position j//NCORES. This balances every layer's work across cores and
    # keeps each core's chunks degree-sorted (tight slot grids).
    nodes = np.arange(N)
    g0 = nodes[in_T0]
    g1 = nodes[in_T2 & ~in_T0]
    g2 = nodes[~in_T2]
    g0 = g0[_group_rank_desc(deg3[g0])]
    g1 = g1[_group_rank_desc(deg2[g1])]
    g2 = g2[_group_rank_desc(deg1[g2])]
    order = np.concatenate([g0, g1, g2])
    j = np.arange(N)
    node_core = np.empty(N, np.int64)
    pos = np.empty(N, np.int64)
    node_core[order] = j % NCORES
    pos[order] = j // NCORES
    n0_k = np.bincount(j[:len(g0)] % NCORES, minlength=NCORES)
    n2_k = np.bincount(j[:len(g0) + len(g1)] % NCORES, minlength=NCORES)
    core_of = node_core[dst]
    p.pos = pos
    p.node_core = node_core

    # --- chunk counts (shared across cores) ---------------------------------
    C1 = NLOC // 128
    C3 = max(1, cdiv(int(n0_k.max()), 128))
    # +1 guarantees the last z2 row of every core is not a T2 member, so it
    # can be explicitly zeroed and used as gather padding.
    C2 = max(C3, cdiv(int(n2_k.max()) + 1, 128))
    C2 = min(C2, C1)
    p.C = [C1, C2, C3]
    # rows of the z table gathered by each layer (per core)
    p.Z = [NLOC, NLOC, C2 * 128]

    # --- per-layer grids -----------------------------------------------------
    # Layer l gathers from a table of NCORES*Z[l] rows; split lo/hi at 32768.
    layers = []
    for li, (emask, Cl) in enumerate([(None, C1), (e2, C2), (e3, C3)]):
        es = src if emask is None else src[emask]
        ed = dst if emask is None else dst[emask]
        ecore = core_of if emask is None else core_of[emask]
        Zl = p.Z[li]
        tab_rows = NCORES * Zl
        assert tab_rows <= 2 * LO, tab_rows
        split = tab_rows > LO
        # source position in the layer's table
        if li < 2:
            srcpos = node_core[es] * NLOC + pos[es]
        else:
            srcpos = node_core[es] * Zl + pos[es]
            assert (pos[es] < Zl).all()
        col = pos[ed]                      # local dst position on owner core
        keep = col < Cl * 128              # always true by construction
        assert keep.all()
        hi_half = srcpos >= LO if split else np.zeros(len(es), bool)

        # per (core, col, half) counts -> shared per-chunk R, capped; the
        # per-column overflow tail goes to dense one-hot blocks instead.
        # per-chunk caps: mean-per-column + 1 sigma (chunks are degree-
        # sorted so early chunks get a higher cap automatically)
        m_lo = np.bincount(col[~hi_half], minlength=Cl * 128)
        m_hi = np.bincount(col[hi_half], minlength=Cl * 128)
        mu_lo = m_lo.reshape(Cl, 128).sum(1) / (128.0 * NCORES)
        mu_hi = m_hi.reshape(Cl, 128).sum(1) / (128.0 * NCORES)
        cap_lo = (np.ceil(mu_lo + 1.0 * np.sqrt(mu_lo))
                  .astype(np.int64) + 1)
        cap_hi = (np.ceil(mu_hi + 1.0 * np.sqrt(mu_hi))
                  .astype(np.int64) + 1)
        Rlo = np.zeros(Cl, np.int64)
        Rhi = np.zeros(Cl, np.int64)
        OVlo = np.zeros(Cl, np.int64)    # overflow blocks per chunk (lo)
        OVhi = np.zeros(Cl, np.int64)
        for k in range(NCORES):
            m = ecore == k
            c_lo = np.bincount(col[m & ~hi_half],
                               minlength=Cl * 128).reshape(Cl, 128)
            c_hi = np.bincount(col[m & hi_half],
                               minlength=Cl * 128).reshape(Cl, 128)
            Rlo = np.maximum(Rlo, np.minimum(c_lo.max(1), cap_lo))
            Rhi = np.maximum(Rhi, np.minimum(c_hi.max(1), cap_hi))
            OVlo = np.maximum(
                OVlo, -(-np.maximum(c_lo - cap_lo[:, None], 0).sum(1) // 128))
            OVhi = np.maximum(
                OVhi, -(-np.maximum(c_hi - cap_hi[:, None], 0).sum(1) // 128))
        layers.append(dict(li=li, Cl=Cl, split=split, srcpos=srcpos, col=col,
                           ecore=ecore, Rlo=Rlo, Rhi=Rhi, Zl=Zl,
                           cap_lo=cap_lo, cap_hi=cap_hi,
                           OVlo=OVlo, OVhi=OVhi))
    p.layers = layers

    # --- gather grouping (chunks packed so one gather covers several) -------
    SLOT_CAP = 104
    for L in layers:
        groups = []
        cur = []
        cur_rows = 0
        for c in range(L["Cl"]):
            r = int(L["Rlo"][c] + L["Rhi"][c] + L["OVlo"][c] + L["OVhi"][c])
            if cur and cur_rows + r > SLOT_CAP:
                groups.append(cur)
                cur, cur_rows = [], 0
            cur.append(c)
            cur_rows += r
        if cur:
            groups.append(cur)
        L["groups"] = groups

    # --- idx stream layout (shared) -----------------------------------------
    # stream = for each layer, for each group: [lo segs per chunk][hi segs]
    off = 0  # in slot-rows of 128
    ovb = 0  # overflow block counter (ovdst columns)
    for L in layers:
        L["lo_off"] = {}
        L["hi_off"] = {}
        L["ovlo_off"] = {}
        L["ovhi_off"] = {}
        L["ovlo_col"] = {}
        L["ovhi_col"] = {}
        for grp in L["groups"]:
            for c in grp:
                L["lo_off"][c] = off
                off += int(L["Rlo"][c])
            for c in grp:
                L["ovlo_off"][c] = off
                L["ovlo_col"][c] = ovb
                off += int(L["OVlo"][c])
                ovb += int(L["OVlo"][c])
            for c in grp:
                L["hi_off"][c] = off
                off += int(L["Rhi"][c])
            for c in grp:
                L["ovhi_off"][c] = off
                L["ovhi_col"][c] = ovb
                off += int(L["OVhi"][c])
                ovb += int(L["OVhi"][c])
    p.total_rows = off                     # total slot rows of 128
    p.WTOT = off * 8                       # wrapped idx columns (16 per row)
    p.NOVB = max(ovb, 1)                   # total overflow blocks

    # --- per-core idx arrays -------------------------------------------------
    pad_lo = [Zl - 1 for Zl in p.Z]                      # core0 fake row
    pad_hi = [NCORES * Zl - 1 - LO for Zl in p.Z]        # core7 fake row
    p.idx = []
    p.ovdst = []
    for k in range(NCORES):
        flat = np.zeros(p.total_rows * 128, np.int16)
        ovd = np.zeros(p.NOVB * 128, np.float32)
        for L in layers:
            li = L["li"]
            m = L["ecore"] == k
            es_pos = L["srcpos"][m]
            ecol = L["col"][m]
            hi_half = es_pos >= LO if L["split"] else np.zeros(m.sum(), bool)
            # rank within (col, half)
            order = np.lexsort((es_pos, ecol, hi_half))
            sc = ecol[order]
            sh = hi_half[order]
            sp = es_pos[order]
            key = sh.astype(np.int64) * (1 << 32) + sc
            newgrp = np.r_[True, np.diff(key) != 0]
            starts = np.flatnonzero(newgrp)
            lens = np.diff(np.r_[starts, len(key)])
            rank = np.arange(len(key)) - np.repeat(starts, lens)
            chunk = sc // 128
            lane = sc % 128
            # fill padding defaults per segment
            for c in range(L["Cl"]):
                for rr, offmap in [(int(L["Rlo"][c]) * 128, "lo_off"),
                                   (int(L["OVlo"][c]) * 128, "ovlo_off")]:
                    if rr:
                        o = L[offmap][c] * 128
                        flat[o:o + rr] = pad_lo[li]
                for rr, offmap in [(int(L["Rhi"][c]) * 128, "hi_off"),
                                   (int(L["OVhi"][c]) * 128, "ovhi_off")]:
                    if rr:
                        o = L[offmap][c] * 128
                        flat[o:o + rr] = pad_hi[li]
            base_lo = np.array([L["lo_off"].get(c, 0) for c in range(L["Cl"])])
            base_hi = np.array([L["hi_off"].get(c, 0) for c in range(L["Cl"])])
            # grid part (rank < cap)
            for half, cap, base, padshift in [
                    (False, L["cap_lo"], base_lo, 0),
                    (True, L["cap_hi"], base_hi, LO)]:
                capv = cap[chunk]
                hm = (sh == half) & (rank < capv)
                if hm.any():
                    sl = (base[chunk[hm]] + rank[hm]) * 128 + lane[hm]
                    flat[sl] = (sp[hm] - padshift).astype(np.int16)
                # overflow part (dense one-hot blocks per chunk)
                om = (sh == half) & (rank >= capv)
                if om.any():
                    ovrank = np.zeros(len(sc), np.int64)
                    # rank within the chunk's overflow stream, by order
                    oidx = np.flatnonzero(om)
                    och = chunk[oidx]
                    osort = np.argsort(och, kind="stable")
                    oo = oidx[osort]
                    cch = chunk[oo]
                    st = np.r_[True, np.diff(cch) != 0]
                    sts = np.flatnonzero(st)
                    ll = np.diff(np.r_[sts, len(cch)])
                    orank = np.arange(len(cch)) - np.repeat(sts, ll)
                    obase = np.array(
                        [L["ovlo_off" if not half else "ovhi_off"].get(c, 0)
                         for c in range(L["Cl"])])
                    cbase = np.array(
                        [L["ovlo_col" if not half else "ovhi_col"].get(c, 0)
                         for c in range(L["Cl"])])
                    slot = obase[cch] * 128 + orank
                    flat[slot] = (sp[oo] - padshift).astype(np.int16)
                    ovslot = cbase[cch] * 128 + orank
                    ovd[ovslot] = (sc[oo] % 128).astype(np.float32)
        wrapped = flat.reshape(-1, 16).T.copy()          # [16, WTOT]
        p.idx.append(np.tile(wrapped, (8, 1)))           # [128, WTOT]
        p.ovdst.append(np.ascontiguousarray(
            ovd.reshape(p.NOVB, 128).T))                 # [128, NOVB]

    # --- per-core xT (feature-major, permuted, fake cols zero) --------------
    p.xT = []
    for k in range(NCORES):
        xp = np.zeros((NLOC, F), np.float32)
        kn = nodes[node_core == k]
        xp[pos[kn]] = x[kn]
        p.xT.append(np.ascontiguousarray(xp.T))          # [F, NLOC]

    # --- per-core pool mask --------------------------------------------------
    p.mask = []
    for k in range(NCORES):
        m = np.zeros(C3 * 128, np.float32)
        m[:n0_k[k]] = 1.0
        p.mask.append(np.broadcast_to(m, (64, C3 * 128)).copy())

    return p


# ----------------------------------------------------------------------------
# Device program
# ----------------------------------------------------------------------------

def build_program(p, W, skip_collectives=False, repeat=1):
    """Emit the Bass/Tile program for one core (SPMD). ``repeat`` emits the
    whole compute body N times (slope-based timing); ``skip_collectives``
    drops collectives (single-core cost-model analysis only)."""
    nc = bacc.Bacc("TRN2")
    NLOC, F = p.NLOC, p.F
    C1, C2, C3 = p.C
    CL = [C1, C2, C3]

    xT_d = nc.dram_tensor("xT", [F, NLOC], F32, kind="ExternalInput")
    idx_d = nc.dram_tensor("idx", [128, p.WTOT], I16, kind="ExternalInput")
    mask_d = nc.dram_tensor("mask", [64, C3 * 128], F32, kind="ExternalInput")
    wr1_d = nc.dram_tensor("W_rel1", [F, 64], F32, kind="ExternalInput")
    wo1_d = nc.dram_tensor("W_root1", [F, 64], F32, kind="ExternalInput")
    wr2_d = nc.dram_tensor("W_rel2", [64, 64], F32, kind="ExternalInput")
    wo2_d = nc.dram_tensor("W_root2", [64, 64], F32, kind="ExternalInput")
    wr3_d = nc.dram_tensor("W_rel3", [64, 64], F32, kind="ExternalInput")
    wo3_d = nc.dram_tensor("W_root3", [64, 64], F32, kind="ExternalInput")
    wfc_d = nc.dram_tensor("W_fc", [64, 10], F32, kind="ExternalInput")
    bfc_d = nc.dram_tensor("b_fc", [1, 10], F32, kind="ExternalInput")
    ovdst_d = nc.dram_tensor("ovdst", [128, p.NOVB], F32,
                             kind="ExternalInput")
    iota_d = nc.dram_tensor("iota", [128, 128], F32, kind="ExternalInput")
    out_d = nc.dram_tensor("probs", [1, 10], F32, kind="ExternalOutput")

    rg = [list(range(NCORES))]

    with tile.TileContext(nc) as tc:
        with (
            tc.tile_pool(name="const", bufs=1) as cpool,
            tc.tile_pool(name="persist", bufs=1) as ppool,
            tc.tile_pool(name="stream", bufs=3) as spool,
            tc.tile_pool(name="gather", bufs=2) as gpool,
            tc.tile_pool(name="psum", bufs=1, space="PSUM") as psum,
            tc.tile_pool(name="dram", bufs=1, space="DRAM") as dram,
        ):
            # ---- constants into SBUF ----
            ident = cpool.tile([128, 128], F32)
            make_identity(nc, ident[:])
            wr1_s = cpool.tile([F, 64], F32, tag="wr1")
            wo1_s = cpool.tile([F, 64], F32, tag="wo1")
            w64 = {}
            for nm, d in [("wr2", wr2_d), ("wo2", wo2_d),
                          ("wr3", wr3_d), ("wo3", wo3_d)]:
                w64[nm] = cpool.tile([64, 64], F32, tag=nm, name=nm)
                nc.sync.dma_start(w64[nm][:], d[:])
            nc.sync.dma_start(wr1_s[:], wr1_d[:])
            nc.sync.dma_start(wo1_s[:], wo1_d[:])
            wfc_s = cpool.tile([64, 10], F32, tag="wfc")
            nc.sync.dma_start(wfc_s[:], wfc_d[:])
            bfc_s = cpool.tile([1, 10], F32, tag="bfc")
            nc.sync.dma_start(bfc_s[:], bfc_d[:])
            mask_s = cpool.tile([64, C3 * 128], F32, tag="mask")
            nc.sync.dma_start(mask_s[:], mask_d[:])
            idx_s = cpool.tile([128, p.WTOT], I16, tag="idx")
            nc.sync.dma_start(idx_s[:], idx_d[:])
            xTall = cpool.tile([F, NLOC], F32, tag="xTall")
            nc.sync.dma_start(xTall[:], xT_d[:])
            zrow = cpool.tile([1, 64], F32, tag="zrow")
            nc.vector.memset(zrow[:], 0.0)
            ovdst_s = cpool.tile([128, p.NOVB], F32, tag="ovdst")
            nc.sync.dma_start(ovdst_s[:], ovdst_d[:])
            iota_s = cpool.tile([128, 128], F32, tag="iota")
            nc.sync.dma_start(iota_s[:], iota_d[:])

            # ---- persistent feature-major activations / roots ----
            hT = [ppool.tile([64, NLOC], F32, tag="h1T", name="h1T"),
                  ppool.tile([64, C2 * 128], F32, tag="h2T", name="h2T"),
                  ppool.tile([64, C3 * 128], F32, tag="h3T", name="h3T")]
            rT = [ppool.tile([64, NLOC], F32, tag="r1T", name="r1T"),
                  ppool.tile([64, C2 * 128], F32, tag="r2T", name="r2T"),
                  ppool.tile([64, C3 * 128], F32, tag="r3T", name="r3T")]

            def store_z_chunks(li, zsrcT, c):
                """z chunk c of table li: W_rel @ zsrcT slice, transpose to
                node-major, DMA to z_own[li]."""
                wrel = wr1_s if li == 0 else w64["wr%d" % (li + 1)]
                sl = slice(c * 128, (c + 1) * 128)
                znT_p = psum.tile([64, 128], F32, tag="znT", bufs=2)
                nc.tensor.matmul(znT_p[:], lhsT=wrel[:], rhs=zsrcT[:, sl],
                                 start=True, stop=True)
                znT_s = spool.tile([64, 128], F32, tag="znT_s")
                nc.scalar.activation(znT_s[:], znT_p[:], ACTF.Copy)
                zn_p = psum.tile([128, 64], F32, tag="zn", bufs=2)
                nc.tensor.transpose(zn_p[:], znT_s[:], ident[:64, :64])
                zn_s = spool.tile([128, 64], F32, tag="zn_s")
                nc.scalar.activation(zn_s[:], zn_p[:], ACTF.Copy)
                nc.sync.dma_start(z_own[li][sl, :], zn_s[:])

            def kick_ag(li):
                zl = p.Z[li]
                nc.gpsimd.dma_start(z_own[li][zl - 1:zl, :], zrow[:])
                if not skip_collectives:
                    nc.gpsimd.collective_compute(
                        "AllGather", AX.bypass, replica_groups=rg,
                        ins=[z_own[li].opt()], outs=[z_tab[li].opt()])

            def root_precompute(li, srcT):
                """rT[li] = W_root.T @ srcT (runs during the AG)."""
                wroot = wo1_s if li == 0 else w64["wo%d" % (li + 1)]
                for c in range(CL[li]):
                    sl = slice(c * 128, (c + 1) * 128)
                    rp = psum.tile([64, 128], F32, tag="znT", bufs=2)
                    nc.tensor.matmul(rp[:], lhsT=wroot[:], rhs=srcT[:, sl],
                                     start=True, stop=True)
                    nc.scalar.activation(rT[li][:, sl], rp[:], ACTF.Copy)

            dbg_stage = int(os.environ.get("GNN_DEBUG_STAGE", "3"))
            for _rep in range(repeat):
                # ---- internal DRAM z tables (per rep: Shared tensors must
                # have a single writer) ----
                z_own = [dram.tile([p.Z[i], 64], F32, name="z%do_%d" % (i, _rep))
                         for i in range(3)]
                z_tab = [dram.tile([NCORES * p.Z[i], 64], F32,
                                   addr_space="Shared",
                                   name="z%dt_%d" % (i, _rep))
                         for i in range(3)]
                pool_in = dram.tile([64, 1], F32, name="pool_in_%d" % _rep)
                pool_out = dram.tile([64, 1], F32, addr_space="Shared",
                                     name="pool_out_%d" % _rep)
                # ---- phase 0: z0 chunks, AG, then r1 during the AG ----
                for c in range(C1):
                    store_z_chunks(0, xTall, c)
                kick_ag(0)
                root_precompute(0, xTall)

                # ---- layers ----
                for li in range(min(3, dbg_stage)):
                    L = p.layers[li]
                    src_tab = z_tab[li]
                    tab_rows = NCORES * p.Z[li]
                    for grp in L["groups"]:
                        rows_lo = sum(int(L["Rlo"][c] + L["OVlo"][c])
                                      for c in grp)
                        rows_hi = sum(int(L["Rhi"][c] + L["OVhi"][c])
                                      for c in grp)
                        rows = rows_lo + rows_hi
                        g = None
                        if rows:
                            g = gpool.tile([128, rows, 64], F32, tag="G",
                                           name="G")
                            if rows_lo:
                                o = L["lo_off"][grp[0]]
                                nc.gpsimd.dma_gather(
                                    g[:, 0:rows_lo, :],
                                    src_tab[0:min(LO, tab_rows), :],
                                    idx_s[:, o * 8:(o + rows_lo) * 8],
                                    rows_lo * 128, rows_lo * 128, 64,
                                    single_packet=False)
                            if rows_hi:
                                o = L["hi_off"][grp[0]]
                                nc.gpsimd.dma_gather(
                                    g[:, rows_lo:rows, :],
                                    src_tab[LO:tab_rows, :],
                                    idx_s[:, o * 8:(o + rows_hi) * 8],
                                    rows_hi * 128, rows_hi * 128, 64,
                                    single_packet=False)

                        for c in grp:
                            sl = slice(c * 128, (c + 1) * 128)
                            rlo = int(L["Rlo"][c])
                            rhi = int(L["Rhi"][c])

                            def fold(a, r):
                                k = r
                                while k > 1:
                                    m = (k + 1) // 2
                                    cnt = k - m
                                    nc.vector.tensor_tensor(
                                        out=g[:, a:a + cnt, :],
                                        in0=g[:, a:a + cnt, :],
                                        in1=g[:, a + m:a + k, :], op=AX.add)
                                    k = m

                            la = L["lo_off"][c] - L["lo_off"][grp[0]]
                            ha = rows_lo + (L["hi_off"][c]
                                            - L["hi_off"][grp[0]])
                            if rlo:
                                fold(la, rlo)
                            if rhi:
                                fold(ha, rhi)
                            if rlo and rhi:
                                nc.vector.tensor_tensor(
                                    out=g[:, la, :], in0=g[:, la, :],
                                    in1=g[:, ha, :], op=AX.add)
                            agg = (g[:, la, :] if rlo else
                                   (g[:, ha, :] if rhi else None))

                            # dense one-hot overflow blocks for this chunk
                            ovblk = []
                            nlo_ov = int(L["OVlo"][c])
                            nhi_ov = int(L["OVhi"][c])
                            for b in range(nlo_ov):
                                grow = (L["ovlo_off"][c]
                                        - L["lo_off"][grp[0]] + b)
                                ovblk.append((grow, L["ovlo_col"][c] + b))
                            for b in range(nhi_ov):
                                grow = (rows_lo + L["ovhi_off"][c]
                                        - L["hi_off"][grp[0]] + b)
                                ovblk.append((grow, L["ovhi_col"][c] + b))

                            if agg is not None or ovblk:
                                aggT_p = psum.tile([64, 128], F32,
                                                   tag="aggT", bufs=3)
                                nmm = (1 if agg is not None else 0) + len(ovblk)
                                i = 0
                                if agg is not None:
                                    nc.tensor.matmul(
                                        aggT_p[:], lhsT=agg, rhs=ident[:],
                                        is_transpose=True, start=True,
                                        stop=(nmm == 1))
                                    i = 1
                                for grow, dcol in ovblk:
                                    sel = spool.tile([128, 128], F32,
                                                     tag="sel", name="sel")
                                    nc.vector.tensor_tensor(
                                        out=sel[:],
                                        in0=ovdst_s[:, dcol:dcol + 1]
                                        .to_broadcast([128, 128]),
                                        in1=iota_s[:], op=AX.is_equal)
                                    nc.tensor.matmul(
                                        aggT_p[:], lhsT=g[:, grow, :],
                                        rhs=sel[:], start=(i == 0),
                                        stop=(i == nmm - 1))
                                    i += 1
                                # h = relu(aggT + rT)
                                nc.vector.tensor_tensor(
                                    out=hT[li][:, sl], in0=aggT_p[:],
                                    in1=rT[li][:, sl], op=AX.add)
                                nc.vector.tensor_scalar_max(
                                    hT[li][:, sl], hT[li][:, sl], 0.0)
                            else:
                                nc.vector.tensor_scalar_max(
                                    hT[li][:, sl], rT[li][:, sl], 0.0)

                            if li < 2 and c * 128 < p.Z[li + 1]:
                                store_z_chunks(li + 1, hT[li], c)

                    if li < 2:
                        kick_ag(li + 1)
                        root_precompute(li + 1, hT[li])

                # ---- pool + fc + softmax ----
                if dbg_stage >= 3:
                    hm = spool.tile([64, C3 * 128], F32, tag="hm")
                    _tail(nc, tc, spool, psum, hm, hT, mask_s, wfc_s,
                          bfc_s, pool_in, pool_out, out_d, p, rg,
                          skip_collectives)
                else:
                    probs_dbg = spool.tile([1, 10], F32, tag="probs_dbg")
                    nc.vector.memset(probs_dbg[:], 0.5)
                    nc.sync.dma_start(out_d[:], probs_dbg[:])

    nc.compile()
    return nc



def _tail(nc, tc, spool, psum, hm, hT, mask_s, wfc_s, bfc_s, pool_in,
          pool_out, out_d, p, rg, skip_collectives=False):
    nc.vector.tensor_tensor(out=hm[:], in0=hT[2][:],
                            in1=mask_s[:], op=AX.mult)
    psum_pool = spool.tile([64, 1], F32, tag="ppart")
    nc.vector.tensor_reduce(psum_pool[:], hm[:],
                            axis=mybir.AxisListType.X, op=AX.add)
    nc.sync.dma_start(pool_in[:], psum_pool[:])
    if not skip_collectives:
        nc.gpsimd.collective_compute(
            "AllReduce", AX.add, replica_groups=rg,
            ins=[pool_in.opt()], outs=[pool_out.opt()])
    pooled = spool.tile([64, 1], F32, tag="pooled")
    nc.sync.dma_start(pooled[:], pool_out[:])
    mean_s = spool.tile([64, 1], F32, tag="mean")
    nc.vector.tensor_scalar_mul(mean_s[:], pooled[:], 1.0 / max(p.n0, 1))
    lg_p = psum.tile([1, 10], F32, tag="lg")
    nc.tensor.matmul(lg_p[:], lhsT=mean_s[:], rhs=wfc_s[:],
                     start=True, stop=True)
    logits = spool.tile([1, 10], F32, tag="logits")
    nc.vector.tensor_tensor(out=logits[:], in0=lg_p[:],
                            in1=bfc_s[:], op=AX.add)
    mx = spool.tile([1, 1], F32, tag="mx")
    nc.vector.tensor_reduce(mx[:], logits[:],
                            axis=mybir.AxisListType.X, op=AX.max)
    nmx = spool.tile([1, 1], F32, tag="nmx")
    nc.vector.tensor_scalar_mul(nmx[:], mx[:], -1.0)
    es = spool.tile([1, 10], F32, tag="es")
    nc.scalar.activation(es[:], logits[:], ACTF.Exp, bias=nmx[:, 0:1])
    ssum = spool.tile([1, 1], F32, tag="ssum")
    nc.vector.tensor_reduce(ssum[:], es[:],
                            axis=mybir.AxisListType.X, op=AX.add)
    inv = spool.tile([1, 1], F32, tag="inv")
    nc.vector.reciprocal(inv[:], ssum[:])
    probs_s = spool.tile([1, 10], F32, tag="probs")
    nc.vector.tensor_scalar_mul(probs_s[:], es[:], inv[:, 0:1])
    nc.sync.dma_start(out_d[:], probs_s[:])

# ----------------------------------------------------------------------------
# Entry point
# ----------------------------------------------------------------------------

def _prep(inputs):
    x = np.ascontiguousarray(np.asarray(inputs["x"], np.float32))
    edge_index = np.asarray(inputs["edge_index"])
    batch = np.asarray(inputs["batch"])
    W = {k: np.ascontiguousarray(np.asarray(inputs[k], np.float32))
         for k in ["W_rel1", "W_root1", "W_rel2", "W_root2",
                   "W_rel3", "W_root3", "W_fc", "b_fc"]}
    p = build_plan(x, edge_index, batch)
    nc = build_program(p, W)
    return nc, _in_maps(p, W)


def _in_maps(p, W):
    in_maps = []
    for k in range(NCORES):
        in_maps.append({
            "xT": p.xT[k], "idx": p.idx[k], "mask": p.mask[k],
            "W_rel1": W["W_rel1"], "W_root1": W["W_root1"],
            "W_rel2": W["W_rel2"], "W_root2": W["W_root2"],
            "W_rel3": W["W_rel3"], "W_root3": W["W_root3"],
            "W_fc": W["W_fc"], "b_fc": W["b_fc"].reshape(1, 10),
            "ovdst": p.ovdst[k],
            "iota": np.tile(np.arange(128, dtype=np.float32), (128, 1)),
        })
    return in_maps


def kernel(**inputs) -> np.ndarray:
    nc, in_maps = _prep(inputs)
    res = run_bass_kernel_spmd(nc, in_maps, list(range(NCORES)))
    return np.asarray(res.results[0]["probs"]).reshape(10).astype(np.float32)

